# revision 45
# baseline (speedup 1.0000x reference)
"""KMLayer (Kuramoto oscillator layer) on 8 Trainium2 NeuronCores via Bass/Tile.

Strategy (row-sharded, output-node parallel), v3:
  - A = sc[0] * conn_w  [N,N] is row-sharded: core r owns rows m in
    [r*M_LOC, (r+1)*M_LOC).  The shard is built once on-device (elementwise
    product of streamed sc/conn_w slabs, split across the DVE and Pool
    engines), transposed through the PE array, and kept RESIDENT in SBUF
    as bf16 A^T [n-partition, m-free] (16 MB/core).
  - The sc/conn_w HBM stream starts at t=0 and the groupnorm/x0 init runs
    underneath it; step-1's matmul for each 256-row m-quarter is issued as
    soon as that quarter of A^T lands, so step 1 hides under the build DMA.
  - Each Euler step: X^T-stationary bf16 matmul over the resident A^T with
    4-way PE column tiling (the four quadrant streams run CONCURRENTLY on
    disjoint PE column tiles - this is the matmul throughput win), a small
    fold-matmul transposes [bc, m] -> [m, bc], then the per-row update
    (tangent projection, omega rotation, pair renormalize) runs on DVE/ACT
    for the local rows only.  The omega term depends only on the previous
    state, so it is computed during the matmul phase.
  - The new local slab (bf16) is AllGather'd across the 8 cores each step;
    the re-gather of the full X into SBUF is split into 4 parallel DMAs so
    descriptor processing spreads across queues and the next step's
    matmuls can start on early chunks.
State is carried in fp32; only the matmul operands (A, gathered X) are bf16.
"""

import numpy as np
import ml_dtypes

import concourse.bass as bass
import concourse.mybir as mybir
import concourse.tile as tile
from concourse import bacc
from concourse.bass_utils import run_bass_kernel_spmd
from concourse.replica_groups import maybe_share_collective_output_space
from concourse.bass_interp import get_hw_module

F32 = mybir.dt.float32
BF16 = mybir.dt.bfloat16
FP8 = mybir.dt.float8e4
ALU = mybir.AluOpType
ACTF = mybir.ActivationFunctionType
AXX = mybir.AxisListType.X

N_CORES = 8
B, C, N_FULL = 2, 16, 8192
BC = B * C  # 32
Q_STEPS = 8
GN_EPS = 1e-5
NRM_EPS = 1e-6


def _bcast(ap, parts):
    """Partition-broadcast view of a [1, f] DRAM AP -> [parts, f]."""
    return bass.AP(tensor=ap.tensor, offset=ap.offset, ap=[[0, parts]] + list(ap.ap[1:]))


def build_program(n=N_FULL, ncores=N_CORES, q_steps=Q_STEPS):
    m_loc = n // ncores            # rows owned per core (1024)
    mch = m_loc // 128             # 128-row chunks per core (8)
    nch = n // 128                 # 128-col contraction chunks (64)
    mq = m_loc // 4                # m-range per PE column-tile group (256)
    fw = mch * BC                  # local elementwise width (256)
    rg = [list(range(ncores))]

    nc = bacc.Bacc("TRN2", target_bir_lowering=False, debug=False,
                   enable_asserts=False, num_devices=ncores)

    # ---- I/O ----
    sc_s = nc.dram_tensor("sc_s", [m_loc, n], F32, kind="ExternalInput").ap()
    cw_s = nc.dram_tensor("cw_s", [m_loc, n], F32, kind="ExternalInput").ap()
    x_nat = nc.dram_tensor("x_nat", [BC, n], F32, kind="ExternalInput").ap()
    c_nat = nc.dram_tensor("c_nat", [BC, n], F32, kind="ExternalInput").ap()
    x_slab = nc.dram_tensor("x_slab", [BC, m_loc], F32, kind="ExternalInput").ap()
    c_slab = nc.dram_tensor("c_slab", [BC, m_loc], F32, kind="ExternalInput").ap()
    gnw_i = nc.dram_tensor("gnw_i", [BC, 1], F32, kind="ExternalInput").ap()
    gnb_i = nc.dram_tensor("gnb_i", [BC, 1], F32, kind="ExternalInput").ap()
    omg_i = nc.dram_tensor("omg_i", [1, mch * BC], F32, kind="ExternalInput").ap()
    gam_i = nc.dram_tensor("gam_i", [1, 1], F32, kind="ExternalInput").ap()
    sel2_i = nc.dram_tensor("sel2_i", [128, BC], F32, kind="ExternalInput").ap()
    id32_i = nc.dram_tensor("id32_i", [32, 32], F32, kind="ExternalInput").ap()
    id128_i = nc.dram_tensor("id128_i", [128, 128], BF16, kind="ExternalInput").ap()
    out_loc = nc.dram_tensor("out_loc", [q_steps, B, m_loc, C], F32,
                             kind="ExternalOutput").ap()

    with tile.TileContext(nc) as tc:
        with tc.tile_pool(name="consts", bufs=1) as consts, \
             tc.tile_pool(name="atbp", bufs=1) as atbp, \
             tc.tile_pool(name="state", bufs=2) as state, \
             tc.tile_pool(name="agd", bufs=2, space="DRAM") as agd, \
             tc.tile_pool(name="psacc", bufs=1, space="PSUM") as psacc:

            # ---------------- constants ----------------
            sel2_sb = consts.tile([128, BC], F32)
            nc.sync.dma_start(out=sel2_sb, in_=sel2_i)
            id32_sb = consts.tile([32, 32], F32)
            nc.sync.dma_start(out=id32_sb, in_=id32_i)
            id128_sb = consts.tile([128, 128], BF16)
            nc.sync.dma_start(out=id128_sb, in_=id128_i)
            gnw_sb = consts.tile([BC, 1], F32)
            nc.sync.dma_start(out=gnw_sb, in_=gnw_i)
            gnb_sb = consts.tile([BC, 1], F32)
            nc.sync.dma_start(out=gnb_sb, in_=gnb_i)
            omg_sb = consts.tile([128, mch * BC], F32)
            nc.sync.dma_start(out=omg_sb, in_=_bcast(omg_i, 128))
            gam_sb = consts.tile([128, 1], F32)
            nc.sync.dma_start(out=gam_sb, in_=_bcast(gam_i, 128))
            eps5_sb = consts.tile([BC, 1], F32)
            nc.vector.memset(eps5_sb, GN_EPS)
            eps6_sb = consts.tile([128, 1], F32)
            nc.vector.memset(eps6_sb, NRM_EPS * NRM_EPS)
            invgam_sb = consts.tile([128, 1], F32)
            nc.vector.reciprocal(out=invgam_sb, in_=gam_sb)

            # warm-up AllGather: the first collective of a given buffer size
            # pays a ~20-30us channel-setup cost; absorb it under the build
            # DMA with a same-size gather on the same agi/ago tag rotation
            # (contents are garbage and unused)
            warm_i = agd.tile([m_loc, BC], FP8, tag="agi")
            ago_space = maybe_share_collective_output_space("AllGather", rg)
            warm_o = agd.tile([n, BC], FP8, tag="ago", addr_space=ago_space)
            nc.gpsimd.collective_compute(
                "AllGather", ALU.bypass, replica_groups=rg,
                ins=[warm_i.opt()], outs=[warm_o.opt()])

            # persistent A^T shard [n_lo=128 part, (n_hi)(m_loc) free] bf16
            atb = atbp.tile([128, nch * m_loc], BF16)
            atb_r = atb.rearrange("p (t m) -> p t m", m=m_loc)

            # psum accumulator: quadrant g -> bank g, partitions 32g..32g+32
            psa = psacc.tile([128, 4, 512], F32)

            def psa_q(g):
                return psa[32 * g:32 * (g + 1), g, 0:mq]

            # state tiles (tags shared with per-step allocations); the
            # gathered state is fp8e4 - the coupling matmul runs with an
            # fp8 stationary (X) against the bf16 moving A^T operand,
            # halving the AllGather payload
            xloc = state.tile([128, mch * BC], F32, tag="xloc")
            xcur = state.tile([128, nch * BC], FP8, tag="xcur")
            y_loc = consts.tile([128, mch * BC], F32)

            def coup_matmuls(xc, chunks, groups=range(4), start=None, stop=None):
                """Quadrant matmuls for contraction chunks; the 4 quadrant
                streams run concurrently on disjoint PE column tiles."""
                xr = xc.rearrange("p (t c) -> p t c", c=BC)
                first, last = chunks[0], chunks[-1]
                for t in chunks:
                    for g in groups:
                        nc.tensor.matmul(
                            psa_q(g),
                            lhsT=xr[:, t, :],
                            rhs=atb_r[:, t, g * mq:(g + 1) * mq],
                            start=(t == first) if start is None else start,
                            stop=(t == last) if stop is None else stop,
                            tile_position=(0, 32 * g))

            def pair_bcast(t, npairs):
                """[128, npairs] -> [128, npairs, 2] view with stride-0 on
                the last dim (each pair-scalar read twice)."""
                return bass.AP(tensor=t.tensor, offset=t.offset,
                               ap=[list(t.ap[0]), [t.ap[1][0], npairs], [0, 2]])

            def pair_normalize(src, npairs, dst_a, dst_b, pool):
                """dst = src / sqrt(||pair||^2 + eps^2); writes dst_a
                (f32 or None) and dst_b (any dtype or None)."""
                sq = pool.tile([128, 2 * npairs], F32, tag="pn_sq")
                nc.vector.tensor_mul(sq, src, src)
                ss = pool.tile([128, npairs], F32, tag="pn_ss")
                nc.vector.tensor_reduce(
                    ss, sq.rearrange("p (g two) -> p g two", two=2),
                    axis=AXX, op=ALU.add)
                nr = pool.tile([128, npairs], F32, tag="pn_nr")
                nc.scalar.activation(out=nr, in_=ss, func=ACTF.Sqrt,
                                     bias=eps6_sb)
                rr = pool.tile([128, npairs], F32, tag="pn_rr")
                nc.vector.reciprocal_approx_fast(out=rr, in_=nr)
                sv = src.rearrange("p (g two) -> p g two", two=2)
                rb = pair_bcast(rr, npairs)
                for dst in (dst_b, dst_a):
                    if dst is None:
                        continue
                    dv = dst.rearrange("p (g two) -> p g two", two=2)
                    nc.vector.tensor_mul(dv, sv, rb)

            # ---------------- init (runs under the build DMA) ----------
            with tc.tile_pool(name="initp", bufs=1) as initp, \
                 tc.tile_pool(name="psinit", bufs=1, space="PSUM") as psinit:

                # -- groupnorm statistics over full c --
                c128 = initp.tile([128, n // 4], F32, tag="ibig")
                nc.sync.dma_start(out=c128,
                                  in_=c_nat.rearrange("a (q m) -> (a q) m", q=4))
                fsub = n // 4
                nsub = 1
                while fsub > 512:
                    assert fsub % 2 == 0
                    fsub //= 2
                    nsub *= 2
                stats = initp.tile([128, nsub, 6], F32)
                c128v = c128.rearrange("p (s m) -> p s m", s=nsub)
                for s in range(nsub):
                    nc.vector.bn_stats(out=stats[:, s, :], in_=c128v[:, s, :])
                mv = initp.tile([128, 2], F32)
                nc.vector.bn_aggr(out=mv, in_=stats)
                # mv[:,1] <- E[x^2] = mean^2 + var
                nc.vector.scalar_tensor_tensor(
                    out=mv[:, 1:2], in0=mv[:, 0:1], scalar=mv[:, 0:1],
                    in1=mv[:, 1:2], op0=ALU.mult, op1=ALU.add)
                ps_s = psinit.tile([32, 2], F32, tag="ps_y")
                nc.tensor.matmul(ps_s, lhsT=sel2_sb, rhs=mv, start=True, stop=True)
                mvg = initp.tile([BC, 2], F32)
                nc.vector.tensor_copy(mvg, ps_s)
                mu2 = initp.tile([BC, 1], F32)
                nc.vector.tensor_mul(mu2, mvg[:, 0:1], mvg[:, 0:1])
                var32 = initp.tile([BC, 1], F32)
                nc.vector.tensor_sub(var32, mvg[:, 1:2], mu2)
                sd32 = initp.tile([BC, 1], F32)
                nc.scalar.activation(out=sd32, in_=var32, func=ACTF.Sqrt,
                                     bias=eps5_sb, scale=1.0)
                rstd = initp.tile([BC, 1], F32)
                nc.vector.reciprocal(out=rstd, in_=sd32)
                scl32 = initp.tile([BC, 1], F32)
                nc.vector.tensor_mul(scl32, rstd, gnw_sb)
                nmu = initp.tile([BC, 1], F32)
                nc.vector.tensor_scalar_mul(nmu, mvg[:, 0:1], -1.0)
                bia32 = initp.tile([BC, 1], F32)
                nc.vector.scalar_tensor_tensor(
                    out=bia32, in0=nmu, scalar=scl32, in1=gnb_sb,
                    op0=ALU.mult, op1=ALU.add)

                # -- y (normalized c) for the local slab, transposed --
                csl = initp.tile([BC, m_loc], F32)
                nc.sync.dma_start(out=csl, in_=c_slab)
                ysl = initp.tile([BC, m_loc], F32)
                nc.scalar.activation(out=ysl, in_=csl, func=ACTF.Identity,
                                     bias=bia32, scale=scl32)
                ps_y = psinit.tile([128, mch * BC], F32, tag="ps_y")
                for mc in range(mch):
                    nc.tensor.transpose(ps_y[:, mc * BC:(mc + 1) * BC],
                                        ysl[:, mc * 128:(mc + 1) * 128], id32_sb)
                nc.vector.tensor_copy(y_loc, ps_y)

                # -- x0: full transposed state + pair-normalize (f32 -> fp8) --
                nquart = 4 if n >= 8192 else 1
                nch_h = nch // nquart
                tpg = min(16, nch_h)  # transposes per psum tile
                for hh in range(nquart):
                    xf = initp.tile([BC, n // nquart], F32, tag="ibig")
                    nc.sync.dma_start(
                        out=xf,
                        in_=x_nat[:, hh * (n // nquart):(hh + 1) * (n // nquart)])
                    x0f = initp.tile([128, nch_h * BC], F32, tag="x0f")
                    for tg in range(nch_h // tpg):
                        ps_x = psinit.tile([128, tpg * BC], F32, tag="ps_x")
                        for tt in range(tpg):
                            t = tg * tpg + tt
                            nc.tensor.transpose(ps_x[:, tt * BC:(tt + 1) * BC],
                                                xf[:, t * 128:(t + 1) * 128], id32_sb)
                        nc.vector.tensor_copy(
                            x0f[:, tg * tpg * BC:(tg + 1) * tpg * BC], ps_x)
                    pair_normalize(x0f, nch_h * BC // 2,
                                   None,
                                   xcur[:, hh * nch_h * BC:(hh + 1) * nch_h * BC],
                                   initp)

                # local x0 (f32) from the per-core slab input
                xsl = initp.tile([BC, m_loc], F32)
                nc.sync.dma_start(out=xsl, in_=x_slab)
                xl_pre = initp.tile([128, mch * BC], F32)
                ps_xl = psinit.tile([128, mch * BC], F32, tag="ps_y")
                for mc in range(mch):
                    nc.tensor.transpose(ps_xl[:, mc * BC:(mc + 1) * BC],
                                        xsl[:, mc * 128:(mc + 1) * 128], id32_sb)
                nc.vector.tensor_copy(xl_pre, ps_xl)
                pair_normalize(xl_pre, mch * BC // 2, xloc, None, initp)

            # ---------------- build + Euler steps ----------------
            # `ew`/`psf` open first so they land in the init pools' freed
            # region (their WAR deps chain to the early init, not to the
            # build); the build pools get fresh space so the sc/cw stream
            # starts at t=0 with no false dependencies.
            omg3 = omg_sb.rearrange("p (g two) -> p g two", two=2)
            with tc.tile_pool(name="ew", bufs=2) as ew, \
                 tc.tile_pool(name="psf", bufs=2, space="PSUM") as psf, \
                 tc.tile_pool(name="bstage", bufs=3) as bstage, \
                 tc.tile_pool(name="bprod", bufs=2) as bprod, \
                 tc.tile_pool(name="pst", bufs=2, space="PSUM") as pst:

              def omega_term(xl3):
                  om = ew.tile([128, fw], F32, tag="om")
                  om3 = om.rearrange("p (g two) -> p g two", two=2)
                  nc.vector.tensor_mul(om3[:, :, 0], xl3[:, :, 1], omg3[:, :, 0])
                  nc.vector.tensor_mul(om3[:, :, 1], xl3[:, :, 0], omg3[:, :, 1])
                  return om

              def bcast_col(t, width):
                  """[128, 1] -> [128, width] stride-0 broadcast view."""
                  return bass.AP(tensor=t.tensor, offset=t.offset,
                                 ap=[list(t.ap[0]), [0, width]])

              # ---------------- build A^T shard (+ fused step-1) ----------
              piece = min(1024, n)
              nhv = 4  # n-quarters per row-chunk (prod tile = 4 KB/partition)
              for j in range(mch):
                  for hv in range(nhv):
                      pr = bprod.tile([128, n // nhv], BF16, tag="prod")
                      for qq in range(n // nhv // piece):
                          q0 = hv * (n // nhv) + qq * piece
                          scp = bstage.tile([128, piece], F32, tag="scp")
                          nc.sync.dma_start(
                              out=scp,
                              in_=sc_s[j * 128:(j + 1) * 128, q0:q0 + piece])
                          cwp = bstage.tile([128, piece], F32, tag="cwp")
                          nc.sync.dma_start(
                              out=cwp,
                              in_=cw_s[j * 128:(j + 1) * 128, q0:q0 + piece])
                          # alternate product engine DVE / Pool; the first
                          # row-chunks go Pool-only (DVE still drains init)
                          if j < 3:
                              eng = nc.gpsimd
                          else:
                              eng = nc.vector if (hv + qq) % 2 == 0 else nc.gpsimd
                          eng.tensor_mul(
                              pr[:, qq * piece:(qq + 1) * piece], scp, cwp)
                      tpg2 = 8
                      nch_v = nch // nhv
                      for tg in range(nch_v // tpg2):
                          pt = pst.tile([128, tpg2 * 128], BF16)
                          for tt in range(tpg2):
                              t = tg * tpg2 + tt
                              nc.tensor.transpose(
                                  pt[:, tt * 128:(tt + 1) * 128],
                                  pr[:, t * 128:(t + 1) * 128], id128_sb)
                          src = pt.rearrange("p (t k) -> p t k", t=tpg2)
                          dst = atb_r[:, hv * nch_v + tg * tpg2:
                                      hv * nch_v + (tg + 1) * tpg2,
                                      j * 128:(j + 1) * 128]
                          nc.scalar.copy(out=dst, in_=src)
                  if j % 2 == 1:
                      # m-quarter j//2 of A^T complete: fold step-1's
                      # matmuls for it under the remaining build DMA
                      coup_matmuls(xcur, list(range(nch)), groups=[j // 2])
                  if j == 5:
                      # late re-sync: peer skew accumulates over the build
                      # and would otherwise be paid serially at step-1's
                      # gather; an async mid-build collective absorbs it
                      # while the DMA stream continues
                      warm2_i = agd.tile([m_loc, BC], FP8, tag="agi")
                      warm2_o = agd.tile([n, BC], FP8, tag="ago",
                                         addr_space=ago_space)
                      nc.gpsimd.collective_compute(
                          "AllGather", ALU.bypass, replica_groups=rg,
                          ins=[warm2_i.opt()], outs=[warm2_o.opt()])

              # end-of-build re-sync: absorbs the last stretch of peer
              # skew so step-1's real gather starts aligned
              warm3_i = agd.tile([m_loc, BC], FP8, tag="agi")
              warm3_o = agd.tile([n, BC], FP8, tag="ago",
                                 addr_space=ago_space)
              nc.gpsimd.collective_compute(
                  "AllGather", ALU.bypass, replica_groups=rg,
                  ins=[warm3_i.opt()], outs=[warm3_o.opt()])

              # step-1's omega and x/gamma terms: only need x0; emitted
              # after the build so they don't head-block the DVE's share
              # of the product stream behind the init chain
              om_pre = omega_term(xloc.rearrange("p (g two) -> p g two", two=2))
              xog_pre = ew.tile([128, fw], F32, tag="xog")
              nc.vector.tensor_mul(xog_pre, xloc, bcast_col(invgam_sb, fw))

              # ---------------- Euler steps ----------------
              for k in range(q_steps):
                  # omega rotation term depends only on the previous state:
                  # compute it during (or before) the matmul phase
                  xl3 = xloc.rearrange("p (g two) -> p g two", two=2)
                  om = om_pre if k == 0 else omega_term(xl3)
                  xog = xog_pre if k == 0 else xog_cur
                  # omx = om + x/gamma, also hidden under the matmul phase
                  omx = ew.tile([128, fw], F32, tag="omx")
                  nc.vector.tensor_add(omx, om, xog)
                  if k > 0:
                      coup_matmuls(xcur, list(range(nch)))
                  # cross-quadrant DVE evictions -> coup.T [32 bc, m_loc]
                  coupT = ew.tile([32, m_loc], F32, tag="coupT")
                  for g in range(4):
                      nc.vector.tensor_copy(coupT[:, g * mq:(g + 1) * mq],
                                            psa_q(g))
                  # PE transposes -> coup [m partitions, bc]
                  psb = psf.tile([128, mch * BC], F32)
                  for mc in range(mch):
                      nc.tensor.transpose(psb[:, mc * BC:(mc + 1) * BC],
                                          coupT[:, mc * 128:(mc + 1) * 128],
                                          id32_sb)
                  # elementwise update on [128, mch*BC], free-dim halves
                  # split across the DVE and Pool engines.  Uses
                  # u = om + yt - tmp + xloc/gamma: xn = normalize(gamma*u)
                  # = normalize(u) (gamma > 0; it is 1.0 in this problem).
                  yt = ew.tile([128, fw], F32, tag="yt")
                  nc.vector.tensor_add(yt, psb, y_loc)
                  pr_t = ew.tile([128, fw], F32, tag="pr_t")
                  sim = ew.tile([128, fw // 2], F32, tag="sim")
                  tmp = ew.tile([128, fw], F32, tag="tmp")
                  poy = ew.tile([128, fw], F32, tag="poy")
                  u = ew.tile([128, fw], F32, tag="u")
                  sq = ew.tile([128, fw], F32, tag="sq")
                  ss = ew.tile([128, fw // 2], F32, tag="ss")
                  hw2 = fw // 2
                  for h, eng in ((0, nc.vector), (1, nc.gpsimd)):
                      fs = slice(h * hw2, (h + 1) * hw2)      # full-width half
                      ps = slice(h * hw2 // 2, (h + 1) * hw2 // 2)  # pair half
                      p3 = lambda t: t[:, fs].rearrange("p (g two) -> p g two",
                                                        two=2)
                      eng.tensor_mul(pr_t[:, fs], xloc[:, fs], yt[:, fs])
                      # pair-sum via strided even+odd add (works on both
                      # DVE and Pool; gpsimd lacks X-axis tensor_reduce)
                      eng.tensor_add(sim[:, ps], p3(pr_t)[:, :, 0],
                                     p3(pr_t)[:, :, 1])
                      eng.tensor_mul(p3(tmp), p3(xloc),
                                     pair_bcast(sim[:, ps], hw2 // 2))
                      eng.tensor_sub(poy[:, fs], yt[:, fs], tmp[:, fs])
                      eng.tensor_add(u[:, fs], poy[:, fs], omx[:, fs])
                      eng.tensor_mul(sq[:, fs], u[:, fs], u[:, fs])
                      eng.tensor_add(ss[:, ps], p3(sq)[:, :, 0],
                                     p3(sq)[:, :, 1])
                  nr = ew.tile([128, fw // 2], F32, tag="pn_nr")
                  nc.scalar.activation(out=nr, in_=ss, func=ACTF.Sqrt,
                                       bias=eps6_sb)
                  rr = ew.tile([128, fw // 2], F32, tag="pn_rr")
                  nc.vector.reciprocal_approx_fast(out=rr, in_=nr)
                  xn = state.tile([128, fw], F32, tag="xloc")
                  xn8 = ew.tile([128, fw], FP8, tag="xn8")
                  u3 = u.rearrange("p (g two) -> p g two", two=2)
                  rb = pair_bcast(rr, fw // 2)
                  nc.vector.tensor_mul(
                      xn8.rearrange("p (g two) -> p g two", two=2), u3, rb)
                  nc.gpsimd.tensor_mul(
                      xn.rearrange("p (g two) -> p g two", two=2), u3, rb)
                  if k < q_steps - 1:
                      # SBUF [p, mh] holds original local row 8p+mh (the
                      # host pre-permutes sc/conn_w rows), so this dump is
                      # contiguous per partition AND lands in DRAM in
                      # natural row order; likewise the regather is one
                      # contiguous-per-partition DMA (host pre-permutes
                      # the A^T / x0 column order to n = 64*p + t)
                      agi = agd.tile([m_loc, BC], FP8, tag="agi")
                      nc.sync.dma_start(
                          out=agi.rearrange("(p mh) c -> p mh c", p=128),
                          in_=xn8.rearrange("p (mh c) -> p mh c", c=BC))
                      ago = agd.tile([n, BC], FP8, tag="ago",
                                     addr_space=ago_space)
                      nc.gpsimd.collective_compute(
                          "AllGather", ALU.bypass, replica_groups=rg,
                          ins=[agi.opt()], outs=[ago.opt()])
                      xnew = state.tile([128, nch * BC], FP8, tag="xcur")
                      xnr = xnew.rearrange("p (t c) -> p t c", c=BC)
                      agr = ago.rearrange("(p t) c -> p t c", p=128)
                      nc.sync.dma_start(out=xnr[:, 0:nch // 2, :],
                                        in_=agr[:, 0:nch // 2, :])
                      nc.scalar.dma_start(out=xnr[:, nch // 2:, :],
                                          in_=agr[:, nch // 2:, :])
                      xcur = xnew
                      # next step's x/gamma, in the collective's shadow
                      xog_cur = ew.tile([128, fw], F32, tag="xog")
                      nc.vector.tensor_mul(xog_cur, xn,
                                           bcast_col(invgam_sb, fw))
                  # stream the step's state slab out (after the gather path)
                  xn4 = xn.rearrange("p (mh b c) -> p mh b c", b=B, c=C)
                  for bb in range(B):
                      nc.sync.dma_start(
                          out=out_loc[k, bb].rearrange("(p mh) c -> p mh c", p=128),
                          in_=xn4[:, :, bb, :])
                  xloc = xn

    nc.compile()
    nc.m = get_hw_module(nc.m)
    return nc


def make_inputs(x, c, sc, gn_w, gn_b, conn_w, omg_param, gamma,
                n=N_FULL, ncores=N_CORES):
    """Host-side marshalling: per-core input dicts."""
    m_loc = n // ncores
    mch = m_loc // 128
    bf16 = ml_dtypes.bfloat16

    x_nat = np.ascontiguousarray(x.reshape(BC, n), dtype=np.float32)
    c_nat = np.ascontiguousarray(c.reshape(BC, n), dtype=np.float32)

    # Marshalling permutations (see kernel comments):
    #  - A rows: marshal row 128*mh+p = original local row 8p+mh, so the
    #    on-chip [p, mh] layout maps to original row 8p+mh and the fp8
    #    state dump lands in DRAM in natural row order.
    #  - A / x0 columns: marshal col 128*t+p = original col 64p+t, so the
    #    gathered state regathers with one contiguous-per-partition DMA.
    rowperm = 8 * (np.arange(m_loc) % 128) + np.arange(m_loc) // 128
    colperm = 64 * (np.arange(n) % 128) + np.arange(n) // 128
    slabperm = 8 * (np.arange(m_loc) % 128) + np.arange(m_loc) // 128
    x_natp = np.ascontiguousarray(x_nat[:, colperm])
    gnw_i = np.ascontiguousarray(np.tile(gn_w.astype(np.float32), B)[:, None])
    gnb_i = np.ascontiguousarray(np.tile(gn_b.astype(np.float32), B)[:, None])

    omg = np.abs(omg_param.astype(np.float32)[:, 0])  # [C//2]
    row = np.empty(BC, np.float32)
    for b in range(B):
        for g in range(C // 2):
            row[b * C + 2 * g] = omg[g]
            row[b * C + 2 * g + 1] = -omg[g]
    omg_i = np.ascontiguousarray(np.tile(row, mch)[None, :])

    gam_i = np.asarray(gamma, np.float32).reshape(1, 1)

    sel2 = np.zeros((128, BC), np.float32)
    for p in range(128):
        for j in range(BC):
            if (p // 4) // 2 == j // 2:
                sel2[p, j] = 1.0 / 8.0
    id32 = np.eye(32, dtype=np.float32)
    id128 = np.eye(128).astype(bf16)

    shared = dict(x_nat=x_natp, c_nat=c_nat, gnw_i=gnw_i, gnb_i=gnb_i,
                  omg_i=omg_i, gam_i=gam_i, sel2_i=sel2,
                  id32_i=id32, id128_i=id128)
    in_maps = []
    for r in range(ncores):
        sl = slice(r * m_loc, (r + 1) * m_loc)
        sc_p = np.asarray(sc[0, sl, :], dtype=np.float32)[rowperm][:, colperm]
        cw_p = np.asarray(conn_w[sl, :], dtype=np.float32)[rowperm][:, colperm]
        in_maps.append(dict(
            shared,
            sc_s=np.ascontiguousarray(sc_p),
            cw_s=np.ascontiguousarray(cw_p),
            x_slab=np.ascontiguousarray(x_nat[:, sl][:, slabperm]),
            c_slab=np.ascontiguousarray(c_nat[:, sl][:, slabperm]),
        ))
    return in_maps


_PROGRAM_CACHE = {}


def get_program(n=N_FULL, ncores=N_CORES, q_steps=Q_STEPS):
    key = (n, ncores, q_steps)
    if key not in _PROGRAM_CACHE:
        _PROGRAM_CACHE[key] = build_program(n, ncores, q_steps)
    return _PROGRAM_CACHE[key]


def kernel(x, c, sc, gn_w, gn_b, conn_w, omg_param, gamma, Q):
    assert int(Q) == Q_STEPS
    x = np.asarray(x); c = np.asarray(c); sc = np.asarray(sc)
    gn_w = np.asarray(gn_w); gn_b = np.asarray(gn_b)
    conn_w = np.asarray(conn_w); omg_param = np.asarray(omg_param)
    gamma = np.asarray(gamma)
    n = x.shape[2]
    nc = get_program(n, N_CORES, Q_STEPS)
    in_maps = make_inputs(x, c, sc, gn_w, gn_b, conn_w, omg_param, gamma,
                          n=n, ncores=N_CORES)
    res = run_bass_kernel_spmd(nc, in_maps, core_ids=list(range(N_CORES)))
    outs = [res.results[r]["out_loc"] for r in range(N_CORES)]
    return np.ascontiguousarray(np.concatenate(outs, axis=2), dtype=np.float32)


# revision 47
# speedup vs baseline: 1.0276x; 1.0276x over previous
"""KMLayer (Kuramoto oscillator layer) on 8 Trainium2 NeuronCores via Bass/Tile.

Strategy (row-sharded, output-node parallel), v3:
  - A = sc[0] * conn_w  [N,N] is row-sharded: core r owns rows m in
    [r*M_LOC, (r+1)*M_LOC).  The shard is built once on-device (elementwise
    product of streamed sc/conn_w slabs, split across the DVE and Pool
    engines), transposed through the PE array, and kept RESIDENT in SBUF
    as bf16 A^T [n-partition, m-free] (16 MB/core).
  - The sc/conn_w HBM stream starts at t=0 and the groupnorm/x0 init runs
    underneath it; step-1's matmul for each 256-row m-quarter is issued as
    soon as that quarter of A^T lands, so step 1 hides under the build DMA.
  - Each Euler step: X^T-stationary bf16 matmul over the resident A^T with
    4-way PE column tiling (the four quadrant streams run CONCURRENTLY on
    disjoint PE column tiles - this is the matmul throughput win), a small
    fold-matmul transposes [bc, m] -> [m, bc], then the per-row update
    (tangent projection, omega rotation, pair renormalize) runs on DVE/ACT
    for the local rows only.  The omega term depends only on the previous
    state, so it is computed during the matmul phase.
  - The new local slab (bf16) is AllGather'd across the 8 cores each step;
    the re-gather of the full X into SBUF is split into 4 parallel DMAs so
    descriptor processing spreads across queues and the next step's
    matmuls can start on early chunks.
State is carried in fp32; only the matmul operands (A, gathered X) are bf16.
"""

import numpy as np
import ml_dtypes

import concourse.bass as bass
import concourse.mybir as mybir
import concourse.tile as tile
from concourse import bacc
from concourse.bass_utils import run_bass_kernel_spmd
from concourse.replica_groups import maybe_share_collective_output_space
from concourse.bass_interp import get_hw_module

F32 = mybir.dt.float32
BF16 = mybir.dt.bfloat16
FP8 = mybir.dt.float8e4
ALU = mybir.AluOpType
ACTF = mybir.ActivationFunctionType
AXX = mybir.AxisListType.X

N_CORES = 8
B, C, N_FULL = 2, 16, 8192
BC = B * C  # 32
Q_STEPS = 8
GN_EPS = 1e-5
NRM_EPS = 1e-6


def _bcast(ap, parts):
    """Partition-broadcast view of a [1, f] DRAM AP -> [parts, f]."""
    return bass.AP(tensor=ap.tensor, offset=ap.offset, ap=[[0, parts]] + list(ap.ap[1:]))


def build_program(n=N_FULL, ncores=N_CORES, q_steps=Q_STEPS):
    m_loc = n // ncores            # rows owned per core (1024)
    mch = m_loc // 128             # 128-row chunks per core (8)
    nch = n // 128                 # 128-col contraction chunks (64)
    mq = m_loc // 4                # m-range per PE column-tile group (256)
    fw = mch * BC                  # local elementwise width (256)
    rg = [list(range(ncores))]

    nc = bacc.Bacc("TRN2", target_bir_lowering=False, debug=False,
                   enable_asserts=False, num_devices=ncores)

    # ---- I/O ----
    sc_s = nc.dram_tensor("sc_s", [m_loc, n], F32, kind="ExternalInput").ap()
    cw_s = nc.dram_tensor("cw_s", [m_loc, n], F32, kind="ExternalInput").ap()
    x_nat = nc.dram_tensor("x_nat", [BC, n], F32, kind="ExternalInput").ap()
    c_nat = nc.dram_tensor("c_nat", [BC, n], F32, kind="ExternalInput").ap()
    x_slab = nc.dram_tensor("x_slab", [BC, m_loc], F32, kind="ExternalInput").ap()
    c_slab = nc.dram_tensor("c_slab", [BC, m_loc], F32, kind="ExternalInput").ap()
    gnw_i = nc.dram_tensor("gnw_i", [BC, 1], F32, kind="ExternalInput").ap()
    gnb_i = nc.dram_tensor("gnb_i", [BC, 1], F32, kind="ExternalInput").ap()
    omg_i = nc.dram_tensor("omg_i", [1, mch * BC], F32, kind="ExternalInput").ap()
    gam_i = nc.dram_tensor("gam_i", [1, 1], F32, kind="ExternalInput").ap()
    sel2_i = nc.dram_tensor("sel2_i", [128, BC], F32, kind="ExternalInput").ap()
    id32_i = nc.dram_tensor("id32_i", [32, 32], F32, kind="ExternalInput").ap()
    id128_i = nc.dram_tensor("id128_i", [128, 128], BF16, kind="ExternalInput").ap()
    out_loc = nc.dram_tensor("out_loc", [q_steps, B, m_loc, C], F32,
                             kind="ExternalOutput").ap()

    with tile.TileContext(nc) as tc:
        with tc.tile_pool(name="consts", bufs=1) as consts, \
             tc.tile_pool(name="atbp", bufs=1) as atbp, \
             tc.tile_pool(name="state", bufs=2) as state, \
             tc.tile_pool(name="agd", bufs=2, space="DRAM") as agd, \
             tc.tile_pool(name="psacc", bufs=1, space="PSUM") as psacc:

            # ---------------- constants ----------------
            sel2_sb = consts.tile([128, BC], F32)
            nc.sync.dma_start(out=sel2_sb, in_=sel2_i)
            id32_sb = consts.tile([32, 32], F32)
            nc.sync.dma_start(out=id32_sb, in_=id32_i)
            id128_sb = consts.tile([128, 128], BF16)
            nc.sync.dma_start(out=id128_sb, in_=id128_i)
            gnw_sb = consts.tile([BC, 1], F32)
            nc.sync.dma_start(out=gnw_sb, in_=gnw_i)
            gnb_sb = consts.tile([BC, 1], F32)
            nc.sync.dma_start(out=gnb_sb, in_=gnb_i)
            omg_sb = consts.tile([128, mch * BC], F32)
            nc.sync.dma_start(out=omg_sb, in_=_bcast(omg_i, 128))
            gam_sb = consts.tile([128, 1], F32)
            nc.sync.dma_start(out=gam_sb, in_=_bcast(gam_i, 128))
            eps5_sb = consts.tile([BC, 1], F32)
            nc.vector.memset(eps5_sb, GN_EPS)
            eps6_sb = consts.tile([128, 1], F32)
            nc.vector.memset(eps6_sb, NRM_EPS * NRM_EPS)
            invgam_sb = consts.tile([128, 1], F32)
            nc.vector.reciprocal(out=invgam_sb, in_=gam_sb)

            # warm-up AllGather: the first collective of a given buffer size
            # pays a ~20-30us channel-setup cost; absorb it under the build
            # DMA with a same-size gather on the same agi/ago tag rotation
            # (contents are garbage and unused)
            warm_i = agd.tile([m_loc, BC], FP8, tag="agi")
            ago_space = maybe_share_collective_output_space("AllGather", rg)
            warm_o = agd.tile([n, BC], FP8, tag="ago", addr_space=ago_space)
            nc.gpsimd.collective_compute(
                "AllGather", ALU.bypass, replica_groups=rg,
                ins=[warm_i.opt()], outs=[warm_o.opt()])

            # persistent A^T shard [n_lo=128 part, (n_hi)(m_loc) free] bf16
            atb = atbp.tile([128, nch * m_loc], BF16)
            atb_r = atb.rearrange("p (t m) -> p t m", m=m_loc)

            # psum accumulator: quadrant g -> bank g, partitions 32g..32g+32
            psa = psacc.tile([128, 4, 512], F32)

            def psa_q(g):
                return psa[32 * g:32 * (g + 1), g, 0:mq]

            # state tiles (tags shared with per-step allocations); the
            # gathered state is fp8e4 - the coupling matmul runs with an
            # fp8 stationary (X) against the bf16 moving A^T operand,
            # halving the AllGather payload
            xloc = state.tile([128, mch * BC], F32, tag="xloc")
            xcur = state.tile([128, nch * BC], FP8, tag="xcur")
            y_loc = consts.tile([128, mch * BC], F32)

            def coup_matmuls(xc, chunks, groups=range(4), start=None, stop=None):
                """Quadrant matmuls for contraction chunks; the 4 quadrant
                streams run concurrently on disjoint PE column tiles."""
                xr = xc.rearrange("p (t c) -> p t c", c=BC)
                first, last = chunks[0], chunks[-1]
                for t in chunks:
                    for g in groups:
                        nc.tensor.matmul(
                            psa_q(g),
                            lhsT=xr[:, t, :],
                            rhs=atb_r[:, t, g * mq:(g + 1) * mq],
                            start=(t == first) if start is None else start,
                            stop=(t == last) if stop is None else stop,
                            tile_position=(0, 32 * g))

            def pair_bcast(t, npairs):
                """[128, npairs] -> [128, npairs, 2] view with stride-0 on
                the last dim (each pair-scalar read twice)."""
                return bass.AP(tensor=t.tensor, offset=t.offset,
                               ap=[list(t.ap[0]), [t.ap[1][0], npairs], [0, 2]])

            def pair_normalize(src, npairs, dst_a, dst_b, pool):
                """dst = src / sqrt(||pair||^2 + eps^2); writes dst_a
                (f32 or None) and dst_b (any dtype or None)."""
                sq = pool.tile([128, 2 * npairs], F32, tag="pn_sq")
                nc.vector.tensor_mul(sq, src, src)
                ss = pool.tile([128, npairs], F32, tag="pn_ss")
                nc.vector.tensor_reduce(
                    ss, sq.rearrange("p (g two) -> p g two", two=2),
                    axis=AXX, op=ALU.add)
                nr = pool.tile([128, npairs], F32, tag="pn_nr")
                nc.scalar.activation(out=nr, in_=ss, func=ACTF.Sqrt,
                                     bias=eps6_sb)
                rr = pool.tile([128, npairs], F32, tag="pn_rr")
                nc.vector.reciprocal_approx_fast(out=rr, in_=nr)
                sv = src.rearrange("p (g two) -> p g two", two=2)
                rb = pair_bcast(rr, npairs)
                for dst in (dst_b, dst_a):
                    if dst is None:
                        continue
                    dv = dst.rearrange("p (g two) -> p g two", two=2)
                    nc.vector.tensor_mul(dv, sv, rb)

            # ---------------- init (runs under the build DMA) ----------
            with tc.tile_pool(name="initp", bufs=1) as initp, \
                 tc.tile_pool(name="psinit", bufs=1, space="PSUM") as psinit:

                # -- groupnorm statistics over full c --
                c128 = initp.tile([128, n // 4], F32, tag="ibig")
                nc.sync.dma_start(out=c128,
                                  in_=c_nat.rearrange("a (q m) -> (a q) m", q=4))
                fsub = n // 4
                nsub = 1
                while fsub > 512:
                    assert fsub % 2 == 0
                    fsub //= 2
                    nsub *= 2
                stats = initp.tile([128, nsub, 6], F32)
                c128v = c128.rearrange("p (s m) -> p s m", s=nsub)
                for s in range(nsub):
                    nc.vector.bn_stats(out=stats[:, s, :], in_=c128v[:, s, :])
                mv = initp.tile([128, 2], F32)
                nc.vector.bn_aggr(out=mv, in_=stats)
                # mv[:,1] <- E[x^2] = mean^2 + var
                nc.vector.scalar_tensor_tensor(
                    out=mv[:, 1:2], in0=mv[:, 0:1], scalar=mv[:, 0:1],
                    in1=mv[:, 1:2], op0=ALU.mult, op1=ALU.add)
                ps_s = psinit.tile([32, 2], F32, tag="ps_y")
                nc.tensor.matmul(ps_s, lhsT=sel2_sb, rhs=mv, start=True, stop=True)
                mvg = initp.tile([BC, 2], F32)
                nc.vector.tensor_copy(mvg, ps_s)
                mu2 = initp.tile([BC, 1], F32)
                nc.vector.tensor_mul(mu2, mvg[:, 0:1], mvg[:, 0:1])
                var32 = initp.tile([BC, 1], F32)
                nc.vector.tensor_sub(var32, mvg[:, 1:2], mu2)
                sd32 = initp.tile([BC, 1], F32)
                nc.scalar.activation(out=sd32, in_=var32, func=ACTF.Sqrt,
                                     bias=eps5_sb, scale=1.0)
                rstd = initp.tile([BC, 1], F32)
                nc.vector.reciprocal(out=rstd, in_=sd32)
                scl32 = initp.tile([BC, 1], F32)
                nc.vector.tensor_mul(scl32, rstd, gnw_sb)
                nmu = initp.tile([BC, 1], F32)
                nc.vector.tensor_scalar_mul(nmu, mvg[:, 0:1], -1.0)
                bia32 = initp.tile([BC, 1], F32)
                nc.vector.scalar_tensor_tensor(
                    out=bia32, in0=nmu, scalar=scl32, in1=gnb_sb,
                    op0=ALU.mult, op1=ALU.add)

                # -- y (normalized c) for the local slab, transposed --
                csl = initp.tile([BC, m_loc], F32)
                nc.sync.dma_start(out=csl, in_=c_slab)
                ysl = initp.tile([BC, m_loc], F32)
                nc.scalar.activation(out=ysl, in_=csl, func=ACTF.Identity,
                                     bias=bia32, scale=scl32)
                ps_y = psinit.tile([128, mch * BC], F32, tag="ps_y")
                for mc in range(mch):
                    nc.tensor.transpose(ps_y[:, mc * BC:(mc + 1) * BC],
                                        ysl[:, mc * 128:(mc + 1) * 128], id32_sb)
                nc.vector.tensor_copy(y_loc, ps_y)

                # -- x0: full transposed state + pair-normalize (f32 -> fp8) --
                nquart = 4 if n >= 8192 else 1
                nch_h = nch // nquart
                tpg = min(16, nch_h)  # transposes per psum tile
                for hh in range(nquart):
                    xf = initp.tile([BC, n // nquart], F32, tag="ibig")
                    nc.sync.dma_start(
                        out=xf,
                        in_=x_nat[:, hh * (n // nquart):(hh + 1) * (n // nquart)])
                    x0f = initp.tile([128, nch_h * BC], F32, tag="x0f")
                    for tg in range(nch_h // tpg):
                        ps_x = psinit.tile([128, tpg * BC], F32, tag="ps_x")
                        for tt in range(tpg):
                            t = tg * tpg + tt
                            nc.tensor.transpose(ps_x[:, tt * BC:(tt + 1) * BC],
                                                xf[:, t * 128:(t + 1) * 128], id32_sb)
                        nc.vector.tensor_copy(
                            x0f[:, tg * tpg * BC:(tg + 1) * tpg * BC], ps_x)
                    pair_normalize(x0f, nch_h * BC // 2,
                                   None,
                                   xcur[:, hh * nch_h * BC:(hh + 1) * nch_h * BC],
                                   initp)

                # local x0 (f32) from the per-core slab input
                xsl = initp.tile([BC, m_loc], F32)
                nc.sync.dma_start(out=xsl, in_=x_slab)
                xl_pre = initp.tile([128, mch * BC], F32)
                ps_xl = psinit.tile([128, mch * BC], F32, tag="ps_y")
                for mc in range(mch):
                    nc.tensor.transpose(ps_xl[:, mc * BC:(mc + 1) * BC],
                                        xsl[:, mc * 128:(mc + 1) * 128], id32_sb)
                nc.vector.tensor_copy(xl_pre, ps_xl)
                pair_normalize(xl_pre, mch * BC // 2, xloc, None, initp)

            # ---------------- build + Euler steps ----------------
            # `ew`/`psf` open first so they land in the init pools' freed
            # region (their WAR deps chain to the early init, not to the
            # build); the build pools get fresh space so the sc/cw stream
            # starts at t=0 with no false dependencies.
            omg3 = omg_sb.rearrange("p (g two) -> p g two", two=2)
            with tc.tile_pool(name="ew", bufs=2) as ew, \
                 tc.tile_pool(name="psf", bufs=2, space="PSUM") as psf, \
                 tc.tile_pool(name="bstage", bufs=4) as bstage, \
                 tc.tile_pool(name="bprod", bufs=2) as bprod, \
                 tc.tile_pool(name="pst", bufs=2, space="PSUM") as pst:

              def omega_term(xl3):
                  om = ew.tile([128, fw], F32, tag="om")
                  om3 = om.rearrange("p (g two) -> p g two", two=2)
                  nc.vector.tensor_mul(om3[:, :, 0], xl3[:, :, 1], omg3[:, :, 0])
                  nc.vector.tensor_mul(om3[:, :, 1], xl3[:, :, 0], omg3[:, :, 1])
                  return om

              def bcast_col(t, width):
                  """[128, 1] -> [128, width] stride-0 broadcast view."""
                  return bass.AP(tensor=t.tensor, offset=t.offset,
                                 ap=[list(t.ap[0]), [0, width]])

              # ---------------- build A^T shard (+ fused step-1) ----------
              piece = min(1024, n)
              nhv = 4  # n-quarters per row-chunk (prod tile = 4 KB/partition)
              for j in range(mch):
                  for hv in range(nhv):
                      pr = bprod.tile([128, n // nhv], BF16, tag="prod")
                      for qq in range(n // nhv // piece):
                          q0 = hv * (n // nhv) + qq * piece
                          scp = bstage.tile([128, piece], F32, tag="scp")
                          nc.sync.dma_start(
                              out=scp,
                              in_=sc_s[j * 128:(j + 1) * 128, q0:q0 + piece])
                          cwp = bstage.tile([128, piece], F32, tag="cwp")
                          nc.sync.dma_start(
                              out=cwp,
                              in_=cw_s[j * 128:(j + 1) * 128, q0:q0 + piece])
                          # alternate product engine DVE / Pool; the first
                          # row-chunks go Pool-only (DVE still drains init)
                          if j < 3:
                              eng = nc.gpsimd
                          else:
                              eng = nc.vector if (hv + qq) % 2 == 0 else nc.gpsimd
                          eng.tensor_mul(
                              pr[:, qq * piece:(qq + 1) * piece], scp, cwp)
                      tpg2 = 8
                      nch_v = nch // nhv
                      for tg in range(nch_v // tpg2):
                          pt = pst.tile([128, tpg2 * 128], BF16)
                          for tt in range(tpg2):
                              t = tg * tpg2 + tt
                              nc.tensor.transpose(
                                  pt[:, tt * 128:(tt + 1) * 128],
                                  pr[:, t * 128:(t + 1) * 128], id128_sb)
                          src = pt.rearrange("p (t k) -> p t k", t=tpg2)
                          dst = atb_r[:, hv * nch_v + tg * tpg2:
                                      hv * nch_v + (tg + 1) * tpg2,
                                      j * 128:(j + 1) * 128]
                          nc.scalar.copy(out=dst, in_=src)
                  if j % 2 == 1:
                      # m-quarter j//2 of A^T complete: fold step-1's
                      # matmuls for it under the remaining build DMA
                      coup_matmuls(xcur, list(range(nch)), groups=[j // 2])
                  if j == 5:
                      # late re-sync: peer skew accumulates over the build
                      # and would otherwise be paid serially at step-1's
                      # gather; an async mid-build collective absorbs it
                      # while the DMA stream continues
                      warm2_i = agd.tile([m_loc, BC], FP8, tag="agi")
                      warm2_o = agd.tile([n, BC], FP8, tag="ago",
                                         addr_space=ago_space)
                      nc.gpsimd.collective_compute(
                          "AllGather", ALU.bypass, replica_groups=rg,
                          ins=[warm2_i.opt()], outs=[warm2_o.opt()])

              # step-1's omega and x/gamma terms: only need x0; emitted
              # after the build so they don't head-block the DVE's share
              # of the product stream behind the init chain
              om_pre = omega_term(xloc.rearrange("p (g two) -> p g two", two=2))
              xog_pre = ew.tile([128, fw], F32, tag="xog")
              nc.vector.tensor_mul(xog_pre, xloc, bcast_col(invgam_sb, fw))

              # ---------------- Euler steps ----------------
              for k in range(q_steps):
                  # omega rotation term depends only on the previous state:
                  # compute it during (or before) the matmul phase
                  xl3 = xloc.rearrange("p (g two) -> p g two", two=2)
                  om = om_pre if k == 0 else omega_term(xl3)
                  xog = xog_pre if k == 0 else xog_cur
                  # omx = om + x/gamma, also hidden under the matmul phase
                  omx = ew.tile([128, fw], F32, tag="omx")
                  nc.vector.tensor_add(omx, om, xog)
                  if k > 0:
                      coup_matmuls(xcur, list(range(nch)))
                  # cross-quadrant DVE evictions -> coup.T [32 bc, m_loc]
                  coupT = ew.tile([32, m_loc], F32, tag="coupT")
                  for g in range(4):
                      nc.vector.tensor_copy(coupT[:, g * mq:(g + 1) * mq],
                                            psa_q(g))
                  # PE transposes -> coup [m partitions, bc]
                  psb = psf.tile([128, mch * BC], F32)
                  for mc in range(mch):
                      nc.tensor.transpose(psb[:, mc * BC:(mc + 1) * BC],
                                          coupT[:, mc * 128:(mc + 1) * 128],
                                          id32_sb)
                  # elementwise update on [128, mch*BC], free-dim halves
                  # split across the DVE and Pool engines.  Uses
                  # u = om + yt - tmp + xloc/gamma: xn = normalize(gamma*u)
                  # = normalize(u) (gamma > 0; it is 1.0 in this problem).
                  yt = ew.tile([128, fw], F32, tag="yt")
                  nc.vector.tensor_add(yt, psb, y_loc)
                  pr_t = ew.tile([128, fw], F32, tag="pr_t")
                  sim = ew.tile([128, fw // 2], F32, tag="sim")
                  tmp = ew.tile([128, fw], F32, tag="tmp")
                  poy = ew.tile([128, fw], F32, tag="poy")
                  u = ew.tile([128, fw], F32, tag="u")
                  sq = ew.tile([128, fw], F32, tag="sq")
                  ss = ew.tile([128, fw // 2], F32, tag="ss")
                  hw2 = fw // 2
                  for h, eng in ((0, nc.vector), (1, nc.gpsimd)):
                      fs = slice(h * hw2, (h + 1) * hw2)      # full-width half
                      ps = slice(h * hw2 // 2, (h + 1) * hw2 // 2)  # pair half
                      p3 = lambda t: t[:, fs].rearrange("p (g two) -> p g two",
                                                        two=2)
                      eng.tensor_mul(pr_t[:, fs], xloc[:, fs], yt[:, fs])
                      # pair-sum via strided even+odd add (works on both
                      # DVE and Pool; gpsimd lacks X-axis tensor_reduce)
                      eng.tensor_add(sim[:, ps], p3(pr_t)[:, :, 0],
                                     p3(pr_t)[:, :, 1])
                      eng.tensor_mul(p3(tmp), p3(xloc),
                                     pair_bcast(sim[:, ps], hw2 // 2))
                      eng.tensor_sub(poy[:, fs], yt[:, fs], tmp[:, fs])
                      eng.tensor_add(u[:, fs], poy[:, fs], omx[:, fs])
                      eng.tensor_mul(sq[:, fs], u[:, fs], u[:, fs])
                      eng.tensor_add(ss[:, ps], p3(sq)[:, :, 0],
                                     p3(sq)[:, :, 1])
                  nr = ew.tile([128, fw // 2], F32, tag="pn_nr")
                  nc.scalar.activation(out=nr, in_=ss, func=ACTF.Sqrt,
                                       bias=eps6_sb)
                  rr = ew.tile([128, fw // 2], F32, tag="pn_rr")
                  nc.vector.reciprocal_approx_fast(out=rr, in_=nr)
                  xn = state.tile([128, fw], F32, tag="xloc")
                  xn8 = ew.tile([128, fw], FP8, tag="xn8")
                  u3 = u.rearrange("p (g two) -> p g two", two=2)
                  rb = pair_bcast(rr, fw // 2)
                  nc.vector.tensor_mul(
                      xn8.rearrange("p (g two) -> p g two", two=2), u3, rb)
                  nc.gpsimd.tensor_mul(
                      xn.rearrange("p (g two) -> p g two", two=2), u3, rb)
                  if k < q_steps - 1:
                      # SBUF [p, mh] holds original local row 8p+mh (the
                      # host pre-permutes sc/conn_w rows), so this dump is
                      # contiguous per partition AND lands in DRAM in
                      # natural row order; likewise the regather is one
                      # contiguous-per-partition DMA (host pre-permutes
                      # the A^T / x0 column order to n = 64*p + t)
                      agi = agd.tile([m_loc, BC], FP8, tag="agi")
                      nc.sync.dma_start(
                          out=agi.rearrange("(p mh) c -> p mh c", p=128),
                          in_=xn8.rearrange("p (mh c) -> p mh c", c=BC))
                      ago = agd.tile([n, BC], FP8, tag="ago",
                                     addr_space=ago_space)
                      nc.gpsimd.collective_compute(
                          "AllGather", ALU.bypass, replica_groups=rg,
                          ins=[agi.opt()], outs=[ago.opt()])
                      xnew = state.tile([128, nch * BC], FP8, tag="xcur")
                      nc.sync.dma_start(
                          out=xnew.rearrange("p (t c) -> p t c", c=BC),
                          in_=ago.rearrange("(p t) c -> p t c", p=128))
                      xcur = xnew
                      # next step's x/gamma, in the collective's shadow
                      xog_cur = ew.tile([128, fw], F32, tag="xog")
                      nc.vector.tensor_mul(xog_cur, xn,
                                           bcast_col(invgam_sb, fw))
                  # stream the step's state slab out (after the gather path)
                  xn4 = xn.rearrange("p (mh b c) -> p mh b c", b=B, c=C)
                  for bb in range(B):
                      nc.sync.dma_start(
                          out=out_loc[k, bb].rearrange("(p mh) c -> p mh c", p=128),
                          in_=xn4[:, :, bb, :])
                  xloc = xn

    nc.compile()
    nc.m = get_hw_module(nc.m)
    return nc


def make_inputs(x, c, sc, gn_w, gn_b, conn_w, omg_param, gamma,
                n=N_FULL, ncores=N_CORES):
    """Host-side marshalling: per-core input dicts."""
    m_loc = n // ncores
    mch = m_loc // 128
    bf16 = ml_dtypes.bfloat16

    x_nat = np.ascontiguousarray(x.reshape(BC, n), dtype=np.float32)
    c_nat = np.ascontiguousarray(c.reshape(BC, n), dtype=np.float32)

    # Marshalling permutations (see kernel comments):
    #  - A rows: marshal row 128*mh+p = original local row 8p+mh, so the
    #    on-chip [p, mh] layout maps to original row 8p+mh and the fp8
    #    state dump lands in DRAM in natural row order.
    #  - A / x0 columns: marshal col 128*t+p = original col 64p+t, so the
    #    gathered state regathers with one contiguous-per-partition DMA.
    rowperm = 8 * (np.arange(m_loc) % 128) + np.arange(m_loc) // 128
    colperm = 64 * (np.arange(n) % 128) + np.arange(n) // 128
    slabperm = 8 * (np.arange(m_loc) % 128) + np.arange(m_loc) // 128
    x_natp = np.ascontiguousarray(x_nat[:, colperm])
    gnw_i = np.ascontiguousarray(np.tile(gn_w.astype(np.float32), B)[:, None])
    gnb_i = np.ascontiguousarray(np.tile(gn_b.astype(np.float32), B)[:, None])

    omg = np.abs(omg_param.astype(np.float32)[:, 0])  # [C//2]
    row = np.empty(BC, np.float32)
    for b in range(B):
        for g in range(C // 2):
            row[b * C + 2 * g] = omg[g]
            row[b * C + 2 * g + 1] = -omg[g]
    omg_i = np.ascontiguousarray(np.tile(row, mch)[None, :])

    gam_i = np.asarray(gamma, np.float32).reshape(1, 1)

    sel2 = np.zeros((128, BC), np.float32)
    for p in range(128):
        for j in range(BC):
            if (p // 4) // 2 == j // 2:
                sel2[p, j] = 1.0 / 8.0
    id32 = np.eye(32, dtype=np.float32)
    id128 = np.eye(128).astype(bf16)

    shared = dict(x_nat=x_natp, c_nat=c_nat, gnw_i=gnw_i, gnb_i=gnb_i,
                  omg_i=omg_i, gam_i=gam_i, sel2_i=sel2,
                  id32_i=id32, id128_i=id128)
    in_maps = []
    for r in range(ncores):
        sl = slice(r * m_loc, (r + 1) * m_loc)
        sc_p = np.asarray(sc[0, sl, :], dtype=np.float32)[rowperm][:, colperm]
        cw_p = np.asarray(conn_w[sl, :], dtype=np.float32)[rowperm][:, colperm]
        in_maps.append(dict(
            shared,
            sc_s=np.ascontiguousarray(sc_p),
            cw_s=np.ascontiguousarray(cw_p),
            x_slab=np.ascontiguousarray(x_nat[:, sl][:, slabperm]),
            c_slab=np.ascontiguousarray(c_nat[:, sl][:, slabperm]),
        ))
    return in_maps


_PROGRAM_CACHE = {}


def get_program(n=N_FULL, ncores=N_CORES, q_steps=Q_STEPS):
    key = (n, ncores, q_steps)
    if key not in _PROGRAM_CACHE:
        _PROGRAM_CACHE[key] = build_program(n, ncores, q_steps)
    return _PROGRAM_CACHE[key]


def kernel(x, c, sc, gn_w, gn_b, conn_w, omg_param, gamma, Q):
    assert int(Q) == Q_STEPS
    x = np.asarray(x); c = np.asarray(c); sc = np.asarray(sc)
    gn_w = np.asarray(gn_w); gn_b = np.asarray(gn_b)
    conn_w = np.asarray(conn_w); omg_param = np.asarray(omg_param)
    gamma = np.asarray(gamma)
    n = x.shape[2]
    nc = get_program(n, N_CORES, Q_STEPS)
    in_maps = make_inputs(x, c, sc, gn_w, gn_b, conn_w, omg_param, gamma,
                          n=n, ncores=N_CORES)
    res = run_bass_kernel_spmd(nc, in_maps, core_ids=list(range(N_CORES)))
    outs = [res.results[r]["out_loc"] for r in range(N_CORES)]
    return np.ascontiguousarray(np.concatenate(outs, axis=2), dtype=np.float32)
